# revision 37
# baseline (speedup 1.0000x reference)
"""Trainium2 Bass kernel for nn_CrossAttention (B=4, NQ=NK=1024, D=1024, H=16).

Sharding: 8 cores = 4 batches x 2 head-groups (8 heads each). Per core:
  - inputs arrive pre-transposed/sliced on host (free): xqT/xkT/xvT [D, T] fp16,
    Wq/Wk/Wv column slices [D, 512] fp16, Wo row slice [512, D] fp16.
  - Q projection lands in per-head DUPLICATED tiles qd[h] [128, NQ] (both
    partition halves hold q_h's 64 dims).
  - K projection lands in per-head BLOCK-DIAGONAL tiles kbd[h] [128, 8, 128]:
    for ktoken chunk j, rows 0:64 x cols 0:64 hold k_h[:, 128j:128j+64] and
    rows 64:128 x cols 64:128 hold k_h[:, 128j+64:128j+128]; off-diag zero.
  - scores: ONE full 128x128-weight matmul per (head, 128-ktok chunk, 512-q
    chunk): out = kbd.T @ qd gives a per-head contiguous [128 ktok, 512 q]
    scoresT block. Full-width weights keep FWL on and LDWEIGHTS hidden, so
    each MM costs ~N cycles; the zero blocks cost nothing (MM time is
    N-cycles regardless of K/M).
  - exp on ScalarE reads a 2-bank [128, 1024] PSUM window in ONE activation.
  - PV with 128-wide weight reads (head h's 65 V-cols + 63 cols of head h+1
    as junk -> PSUM rows 65..127 are garbage and never read); denominators
    via the ones-column trick (PSUM row 64), reciprocal 2-op approx, gpsimd
    partition_broadcast, DVE mul.
  - out-projection contracts head-pairs; host sums the two head-group
    partials per batch and adds the bias.
All matmuls fp16 (1 cycle/row on PE), accumulation fp32 in PSUM.
Emission is software-pipelined at fine grain: scores chunks (which feed the
ScalarE exp stream) are interleaved with projection / PV / out-proj "filler"
units so the PE stream never blocks on ScalarE.
"""
import sys

sys.path.insert(0, "/opt/trn_rl_repo")

from collections import deque
from contextlib import ExitStack

import numpy as np

import concourse.bass as bass
import concourse.tile as tile
from concourse import bacc, mybir
from concourse.bass_utils import run_bass_kernel_spmd

F32 = mybir.dt.float32
F16 = mybir.dt.float16

B, NQ, NK, D, H, HD = 4, 1024, 1024, 1024, 16, 64
NCORES = 8
HPC = 8          # heads per core
F = HPC * HD     # 512: per-core projection width
KT = D // 128    # 8 k-tiles over D
PAIRS = HPC // 2  # 4 head pairs
TKT = NK // 128  # 8 tiles over key tokens
NCH = NQ // 512  # 2 moving chunks over query tokens
VW = HD + 1      # 65: V cols per head incl ones column
VPW = HPC * VW + 63  # vp row width padded so every head has 128 readable cols


def _emit(tc):
    nc = tc.nc
    ctx = ExitStack()

    xqT = nc.dram_tensor("xqT", [D, NQ], F16, kind="ExternalInput").ap()
    xkT = nc.dram_tensor("xkT", [D, NK], F16, kind="ExternalInput").ap()
    xvT = nc.dram_tensor("xvT", [D, NK], F16, kind="ExternalInput").ap()
    wq = nc.dram_tensor("wq", [D, F], F16, kind="ExternalInput").ap()
    wk = nc.dram_tensor("wk", [D, F], F16, kind="ExternalInput").ap()
    wv = nc.dram_tensor("wv", [D, F], F16, kind="ExternalInput").ap()
    wo = nc.dram_tensor("wo", [F, D], F16, kind="ExternalInput").ap()
    out = nc.dram_tensor("out", [NQ, D], F16, kind="ExternalOutput").ap()

    wpool = ctx.enter_context(tc.tile_pool(name="wpool", bufs=1))
    qkv = ctx.enter_context(tc.tile_pool(name="qkv", bufs=1))
    # static pool for the 24 input-stream tiles, ring pool for exp tiles
    xpool = ctx.enter_context(tc.tile_pool(name="xpool", bufs=1))
    expool = ctx.enter_context(tc.tile_pool(name="expool", bufs=26))
    psum = ctx.enter_context(tc.tile_pool(name="psum", bufs=4, space="PSUM"))
    psum2 = ctx.enter_context(tc.tile_pool(name="psum2", bufs=2, space="PSUM"))
    nrm = ctx.enter_context(tc.tile_pool(name="nrm", bufs=1))
    ost = ctx.enter_context(tc.tile_pool(name="ost", bufs=2))

    # ---- persistent weights: DMA issue spread across otherwise-idle queues
    wq_sb = wpool.tile([128, KT, F], F16, tag="wq")
    wk_sb = wpool.tile([128, KT, F], F16, tag="wk")
    wv_sb = wpool.tile([128, KT, F], F16, tag="wv")
    wo_sb = wpool.tile([128, PAIRS, D], F16, tag="wo")
    # DMA priority: pair-0 scores need only xq, xk and the pair-0 weight
    # columns (~4.5MB of the 9.25MB input), so wq/wk stream per pair and
    # xq/xk per k-tile (projection MMs pipeline with tile arrival). The
    # rest (xv, wv, wo, later pairs) trails. Two rings (sync, gpsimd), each
    # in consumption order; scalar queue stays clear for the exp stream.
    nc.sync.dma_start(
        out=wq_sb[:, :, 0:128],
        in_=wq[:, 0:128].rearrange("(k p) c -> p k c", k=KT))
    nc.gpsimd.dma_start(
        out=wk_sb[:, :, 0:128],
        in_=wk[:, 0:128].rearrange("(k p) c -> p k c", k=KT))

    # ---- persistent intermediates ----
    qd = [qkv.tile([128, NQ], F16, tag=f"qd{h}", name=f"qd{h}")
          for h in range(HPC)]
    kbd = [qkv.tile([128, TKT, 128], F16, tag=f"kbd{h}", name=f"kbd{h}")
           for h in range(HPC)]
    vp_sb = qkv.tile([128, TKT, VPW], F16, tag="vp")
    att = [qkv.tile([128, NQ], F16, tag=f"att{p}", name=f"att{p}")
           for p in range(PAIRS)]
    # zero the block-diag K tiles once: pair 0's on the DVE front (needed in
    # ~10us), the rest on the otherwise-idle gpsimd engine. V needs no bulk
    # zeroing (every head's 65 cols are fully written); just the ones column
    # and the 63-col padding tail after head 7.
    for h in range(2):
        nc.vector.memset(kbd[h][:], 0.0)
    vp4 = vp_sb[:, :, 0:HPC * VW].rearrange("p t (h d) -> p t h d", h=HPC)
    nc.vector.memset(vp4[:, :, :, HD:VW], 1.0)
    nc.vector.memset(vp_sb[:, :, HPC * VW:VPW], 0.0)

    xq_sb = xpool.tile([128, KT, NQ], F16, tag="xq", name="xq_sb")
    xk_sb = xpool.tile([128, KT, NK], F16, tag="xk", name="xk_sb")
    xv_sb = xpool.tile([128, KT, NK], F16, tag="xv", name="xv_sb")
    # pair-0's inputs split across ALL THREE rings (~1.5MB each) — the
    # scalar ring is idle until the exp stream starts anyway
    for k in range(KT):
        eng = (nc.sync if k < 5 else nc.scalar)
        eng.dma_start(out=xq_sb[:, k, :], in_=xqT[k * 128:(k + 1) * 128, :])
        eng = (nc.gpsimd if k < 5 else nc.scalar)
        eng.dma_start(out=xk_sb[:, k, :], in_=xkT[k * 128:(k + 1) * 128, :])
    # later-pair weights then the V-side bulk trail pair-0's inputs on the
    # same two rings (ring order = issue order, so pair-0's 4.5MB gets the
    # full HBM bandwidth first). Latency-critical mid-stream DMAs (qd dups)
    # ride the otherwise-clear scalar ring instead.
    for m in (1, 2, 3):
        nc.sync.dma_start(
            out=wq_sb[:, :, m * 128:(m + 1) * 128],
            in_=wq[:, m * 128:(m + 1) * 128].rearrange("(k p) c -> p k c", k=KT))
        nc.gpsimd.dma_start(
            out=wk_sb[:, :, m * 128:(m + 1) * 128],
            in_=wk[:, m * 128:(m + 1) * 128].rearrange("(k p) c -> p k c", k=KT))
    nc.sync.dma_start(out=xv_sb[:], in_=xvT.rearrange("(k p) t -> p k t", k=KT))
    nc.gpsimd.dma_start(out=wv_sb[:], in_=wv.rearrange("(k p) f -> p k f", k=KT))
    nc.gpsimd.dma_start(out=wo_sb[:], in_=wo.rearrange("(q p) d -> p q d", q=PAIRS))
    xq_t = [xq_sb[:, k, :] for k in range(KT)]
    xk_t = [xk_sb[:, k, :] for k in range(KT)]
    xv_t = [xv_sb[:, k, :] for k in range(KT)]
    # later pairs' block-diag zeroing on gpsimd, queued behind its DMA issues
    for h in range(2, HPC):
        nc.gpsimd.memset(kbd[h][:], 0.0)

    scale = 1.0 / float(np.sqrt(HD))
    ex = {}
    pv_ps = {}

    # ---------------- emission units ----------------
    def emit_qproj(m, n):
        """Q projection psum group -> duplicated per-head tiles (lower half)."""
        ps = psum.tile([128, 512], F32, tag="ps", name=f"ps_q{m}_{n}")
        for k in range(KT):
            nc.tensor.matmul(out=ps[:],
                             lhsT=wq_sb[:, k, m * 128:(m + 1) * 128],
                             rhs=xq_t[k][:, n * 512:(n + 1) * 512],
                             start=(k == 0), stop=(k == KT - 1))
        nco = slice(n * 512, (n + 1) * 512)
        nc.vector.tensor_copy(out=qd[2 * m][0:64, nco], in_=ps[0:64, :])
        nc.vector.tensor_copy(out=qd[2 * m + 1][0:64, nco], in_=ps[64:128, :])
        # replicate the 64 head dims into the upper partition half, per
        # q-chunk so scores n=0 can start before n=1's projection lands
        nc.scalar.dma_start(out=qd[2 * m][64:128, nco], in_=qd[2 * m][0:64, nco])
        nc.scalar.dma_start(out=qd[2 * m + 1][64:128, nco],
                            in_=qd[2 * m + 1][0:64, nco])

    def emit_kproj(m, n):
        """K projection psum group -> block-diagonal per-head tiles."""
        ps = psum.tile([128, 512], F32, tag="ps", name=f"ps_k{m}_{n}")
        for k in range(KT):
            nc.tensor.matmul(out=ps[:],
                             lhsT=wk_sb[:, k, m * 128:(m + 1) * 128],
                             rhs=xk_t[k][:, n * 512:(n + 1) * 512],
                             start=(k == 0), stop=(k == KT - 1))
        psr = ps[:].rearrange("p (c two s) -> p c two s", c=4, two=2)
        jc = slice(n * 4, (n + 1) * 4)
        for hh in range(2):
            h = 2 * m + hh
            rows = slice(hh * 64, (hh + 1) * 64)
            nc.vector.tensor_copy(out=kbd[h][0:64, jc, 0:64],
                                  in_=psr[rows, :, 0, :])
            nc.vector.tensor_copy(out=kbd[h][64:128, jc, 64:128],
                                  in_=psr[rows, :, 1, :])

    def emit_vproj(half, t4):
        """one V projection psum group: token chunk half*512 + t4*128."""
        psv = psum.tile([128, 512], F32, tag="ps", name=f"psv_{half}_{t4}")
        for k in range(KT):
            nc.tensor.matmul(out=psv[:],
                             lhsT=xv_t[k][:, half * 512 + t4 * 128:
                                          half * 512 + (t4 + 1) * 128],
                             rhs=wv_sb[:, k, :], start=(k == 0),
                             stop=(k == KT - 1))
        tk = half * 4 + t4
        nc.vector.tensor_copy(
            out=vp4[:, tk, :, 0:HD],
            in_=psv[:].rearrange("p (h d) -> p h d", h=HPC))

    def emit_scores_chunk(h, j):
        """scoresT + exp for head h, ktoken chunk j: one full-width matmul
        per 512-q chunk into a 2-bank psum, one [128,1024] exp."""
        sps = psum2.tile([128, 1024], F32, tag="sps", name=f"sps_{h}_{j}")
        for n in range(NCH):
            nc.tensor.matmul(out=sps[:, n * 512:(n + 1) * 512],
                             lhsT=kbd[h][:, j, :],
                             rhs=qd[h][:, n * 512:(n + 1) * 512],
                             start=True, stop=True)
        t = expool.tile([128, NQ], F16, tag="ex", name=f"ex_{h}_{j}")
        nc.scalar.activation(out=t[:], in_=sps[:],
                             func=mybir.ActivationFunctionType.Exp, scale=scale)
        ex[(h, j)] = t

    def emit_pv_unit(h, n):
        """PV accumulation for head h, q-chunk n. 128-wide weight read keeps
        FWL on; PSUM rows 65..127 are junk from the next head's cols."""
        pspv = psum.tile([128, 512], F32, tag="ps", name=f"pv_{h}_{n}")
        for k in range(TKT):
            nc.tensor.matmul(out=pspv[:],
                             lhsT=vp_sb[:, k, h * VW:h * VW + 128],
                             rhs=ex[(h, k)][:, n * 512:(n + 1) * 512],
                             start=(k == 0), stop=(k == TKT - 1))
        pv_ps[(h % 2, n, h // 2)] = pspv

    def emit_norm(p):
        """normalization for head pair p (reads pv_ps of both heads)."""
        # denominators: collect rows 64 at partitions 0/32 (legal start
        # partitions), one 1-op approx reciprocal for both heads (~1e-4
        # relative, far inside the tolerance)
        den = nrm.tile([33, NQ], F32, tag="den", name=f"den_{p}")
        nc.vector.memset(den[:], 1.0)
        for hh in range(2):
            for n in range(NCH):
                nc.vector.tensor_copy(out=den[hh * 32:hh * 32 + 1,
                                              n * 512:(n + 1) * 512],
                                      in_=pv_ps[(hh, n, p)][64:65, :])
        rec = nrm.tile([33, NQ], F32, tag="rec", name=f"rec_{p}")
        nc.vector.reciprocal_approx_fast(out=rec[:], in_=den[:])
        # HW partition_broadcast reads physical partition 0 (ignores the AP
        # base partition), so move head 1's reciprocal row down first
        rec1 = nrm.tile([1, NQ], F32, tag="rec1", name=f"rec1_{p}")
        nc.sync.dma_start(out=rec1[:], in_=rec[32:33, :])
        for hh in range(2):
            rb = nrm.tile([64, NQ], F32, tag="rb", name=f"rb_{p}_{hh}")
            nc.gpsimd.partition_broadcast(out_ap=rb[:],
                                          in_ap=(rec[0:1, :] if hh == 0
                                                 else rec1[:]),
                                          channels=64)
            if hh == 0:
                for n in range(NCH):
                    nc.vector.tensor_mul(out=att[p][0:64, n * 512:(n + 1) * 512],
                                         in0=pv_ps[(hh, n, p)][0:64, :],
                                         in1=rb[:, n * 512:(n + 1) * 512])
            else:
                tmp = nrm.tile([64, NQ], F16, tag="tmp", name=f"tmp_{p}")
                for n in range(NCH):
                    nc.vector.tensor_mul(out=tmp[:, n * 512:(n + 1) * 512],
                                         in0=pv_ps[(hh, n, p)][0:64, :],
                                         in1=rb[:, n * 512:(n + 1) * 512])
                nc.sync.dma_start(out=att[p][64:128, :], in_=tmp[:])

    def emit_outproj_chain(q, n):
        pso = psum.tile([128, 512], F32, tag="ps", name=f"pso_{q}_{n}")
        for p4 in range(PAIRS):
            nc.tensor.matmul(out=pso[:],
                             lhsT=att[p4][:, q * 128:(q + 1) * 128],
                             rhs=wo_sb[:, p4, n * 512:(n + 1) * 512],
                             start=(p4 == 0), stop=(p4 == PAIRS - 1))
        ot = ost.tile([128, 512], F16, tag="ot", name=f"ot_{q}_{n}")
        nc.scalar.copy(out=ot[:], in_=pso[:])
        eng = nc.sync if (q + n) % 2 == 0 else nc.gpsimd
        eng.dma_start(out=out[q * 128:(q + 1) * 128,
                              n * 512:(n + 1) * 512], in_=ot[:])

    # ---------------- schedule ----------------
    # pair-0 projections up front so the exp stream starts ASAP
    for n in range(NCH):
        emit_qproj(0, n)
    for n in range(NCH):
        emit_kproj(0, n)

    # filler queue: PE work units (~1.7us each) interleaved between scores
    # chunks so the PE keeps running while ScalarE chews the exp stream.
    # pair-1 projections first (needed by scores h2), then V (needed by
    # PV(h0) mid-h2), then the remaining projections.
    fillq = deque()
    for n in range(NCH):
        fillq.append(("q", 1, lambda n=n: emit_qproj(1, n)))
    for n in range(NCH):
        fillq.append(("k", 1, lambda n=n: emit_kproj(1, n)))
    for half in range(2):
        for t4 in range(4):
            fillq.append(("v", None,
                          lambda half=half, t4=t4: emit_vproj(half, t4)))
    for m in (2, 3):
        for n in range(NCH):
            fillq.append(("q", m, lambda m=m, n=n: emit_qproj(m, n)))
        for n in range(NCH):
            fillq.append(("k", m, lambda m=m, n=n: emit_kproj(m, n)))

    proj_done = {0}

    def force_proj(m):
        """pop + emit any still-queued projection units for pair m."""
        if m in proj_done:
            return
        keep = deque()
        while fillq:
            kind, mm, fn = fillq.popleft()
            if kind in ("q", "k") and mm == m:
                fn()
            else:
                keep.append((kind, mm, fn))
        fillq.extend(keep)
        proj_done.add(m)

    # PV(h) emission point: lag 2 heads early on (V proj is still in the
    # filler queue), lag 1 from h4 so the tail stays short.
    pv_at = {2: [0], 3: [1], 4: [2], 5: [3, 4], 6: [5], 7: [6]}

    for h in range(HPC):
        force_proj(h // 2)
        for j in range(TKT):
            emit_scores_chunk(h, j)
            if j % 2 == 1 and fillq:
                fillq.popleft()[2]()
        for hp in pv_at.get(h, ()):
            for n in range(NCH):
                emit_pv_unit(hp, n)
            if hp % 2 == 1:
                emit_norm(hp // 2)

    while fillq:
        fillq.popleft()[2]()
    for n in range(NCH):
        emit_pv_unit(HPC - 1, n)
    emit_norm(PAIRS - 1)

    # ---- output projection (p4=0..2 of each chain can run during norm3) ----
    for q in range(NQ // 128):
        for n in range(NCH):
            emit_outproj_chain(q, n)
    ctx.close()


_NC_CACHE = None


def build():
    global _NC_CACHE
    if _NC_CACHE is None:
        nc = bacc.Bacc("TRN2", target_bir_lowering=False, debug=False,
                       num_devices=NCORES)
        with tile.TileContext(nc) as tc:
            _emit(tc)
        nc.compile()
        _NC_CACHE = nc
    return _NC_CACHE


def make_in_maps(inputs):
    q = np.asarray(inputs["query_tokens"], dtype=np.float32)
    kk = np.asarray(inputs["key_tokens"], dtype=np.float32)
    v = np.asarray(inputs["value_tokens"], dtype=np.float32)
    Wq = np.asarray(inputs["Wq"], dtype=np.float32)
    Wk = np.asarray(inputs["Wk"], dtype=np.float32)
    Wv = np.asarray(inputs["Wv"], dtype=np.float32)
    Wo = np.asarray(inputs["Wo"], dtype=np.float32)

    qT = [np.ascontiguousarray(q[b].T).astype(np.float16) for b in range(B)]
    kT = [np.ascontiguousarray(kk[b].T).astype(np.float16) for b in range(B)]
    vT = [np.ascontiguousarray(v[b].T).astype(np.float16) for b in range(B)]
    wq_g = [np.ascontiguousarray(Wq[:, g * F:(g + 1) * F]).astype(np.float16)
            for g in range(2)]
    wk_g = [np.ascontiguousarray(Wk[:, g * F:(g + 1) * F]).astype(np.float16)
            for g in range(2)]
    wv_g = [np.ascontiguousarray(Wv[:, g * F:(g + 1) * F]).astype(np.float16)
            for g in range(2)]
    wo_g = [np.ascontiguousarray(Wo[g * F:(g + 1) * F, :]).astype(np.float16)
            for g in range(2)]

    in_maps = []
    for c in range(NCORES):
        b, g = c // 2, c % 2
        in_maps.append({
            "xqT": qT[b], "xkT": kT[b], "xvT": vT[b],
            "wq": wq_g[g], "wk": wk_g[g], "wv": wv_g[g], "wo": wo_g[g],
        })
    return in_maps


def combine(results, bo):
    out = np.zeros((B, NQ, D), dtype=np.float32)
    for c in range(NCORES):
        out[c // 2] += results[c]["out"]
    out += np.asarray(bo, dtype=np.float32)[None, None, :]
    return out


def kernel(**inputs):
    nc = build()
    in_maps = make_in_maps(inputs)
    res = run_bass_kernel_spmd(nc, in_maps, list(range(NCORES)))
    return combine(res.results, inputs["bo"])
